# revision 18
# baseline (speedup 1.0000x reference)
"""Fused LayerNorm + causal multi-head attention for Trainium2, 8 NeuronCores.

Problem: x[2,2048,1024] -> LN -> qkv proj (w_qkv[1024,3072]) -> 16-head causal
attention (d=64) -> out proj (w_out[1024,1024]).

Sharding (no cross-core communication):
  core c = b*4 + hg   (b in {0,1} batches, hg in {0..3} head-groups of 4 heads)
  Each core computes its batch's LN + its 4 heads' qkv/attention + a partial
  out-projection (its 256 rows of w_out). Host sums the 4 partials per batch.

Device algorithm (transposed layout: features on partitions, sequence on the
free axis; all matmuls in float32r = full-speed fp32 on the PE):
  1. stats: colsums of xT and xT^2 via ones-matmuls -> mean/std/rs rows
  2. LN folded into the QKV matmul: W premultiplied by ln_w (and SCALE for q)
     on host; the (-mean*u + std*vb) correction enters as 2 extra contraction
     rows; the rs[n] factor is one broadcast multiply on the psum epilogue
  3. qkvT[768,2048] tiles = W.T @ xT; v transposed to natural layout via PE
     transpose, with appended ones columns for the softmax denominator
  4. per head, per 512-wide i-block: sT[j,i] = kT.T@qT tiles, exp on ScalarE
     (no max subtraction: scores are O(6) for this distribution), causal
     masking assembled on GpSimd, PV + denominator accumulated as [66,512]
     psum, normalized by the reciprocal denominator broadcast via K=1 matmul
  5. partial out-proj [2048,1024] = oT.T @ w_out rows, DMA out
"""
import os
import sys

for _p in ("/opt/trn_rl_repo", "/root/.axon_site/_ro/trn_rl_repo"):
    if os.path.isdir(_p) and _p not in sys.path:
        sys.path.insert(0, _p)

import numpy as np

import concourse.bass as bass  # noqa: F401  (import side effects / debugging)
import concourse.mybir as mybir
import concourse.tile as tile
from concourse import bacc
from concourse.bass_utils import run_bass_kernel_spmd

F32 = mybir.dt.float32
F32R = mybir.dt.float32r
MUL = mybir.AluOpType.mult
ADD = mybir.AluOpType.add
AF = mybir.ActivationFunctionType

B, N, DIM = 2, 2048, 1024
HEADS, DH = 16, 64
HPC = 4            # heads per core
CD = HPC * DH      # 256 output channels per core
SCALE = DH ** -0.5
EPS = 1e-5
NT = N // 512      # 4 col-blocks of 512
NK = DIM // 128    # 8 contraction chunks
NROW = N // 128    # 16 row tiles of 128
VW = DH + 2        # 66: v columns + 2 ones columns per head


def _r32(x, bits=13):
    """tf32-style rounding: round-to-nearest, drop low `bits` mantissa bits."""
    v = np.ascontiguousarray(x, dtype=np.float32).view(np.uint32)
    v = (v + (1 << (bits - 1))) & np.uint32(~((1 << bits) - 1) & 0xFFFFFFFF)
    return v.view(np.float32)


def _build():
    nc = bacc.Bacc("TRN2", target_bir_lowering=False, debug=False)

    xT_ext = nc.declare_dram_parameter("xT", [DIM, N], F32R, isOutput=False)
    w_ext = nc.declare_dram_parameter("wqkv", [DIM, 3 * CD], F32R, isOutput=False)
    uv_ext = nc.declare_dram_parameter("uv", [2, 3 * CD], F32R, isOutput=False)
    wo_ext = nc.declare_dram_parameter("wout", [CD, DIM], F32R, isOutput=False)
    ones_ext = nc.declare_dram_parameter("ones", [128, 128], F32R, isOutput=False)
    mask_ext = nc.declare_dram_parameter("mask", [128, 128], F32R, isOutput=False)
    id_ext = nc.declare_dram_parameter("ident", [128, 128], F32R, isOutput=False)
    out_ext = nc.declare_dram_parameter("out", [N, DIM], F32, isOutput=True)

    with tile.TileContext(nc) as tc:
        with (
            nc.allow_low_precision(reason="float32r is 4-byte; psum stays f32"),
            tc.tile_pool(name="persist", bufs=1) as pp,
            tc.tile_pool(name="small", bufs=1) as sp,
        ):
            ones_t = pp.tile([128, 128], F32R, tag="ones")
            mask_t = pp.tile([128, 128], F32R, tag="mask")
            ident_t = pp.tile([128, 128], F32R, tag="ident")
            nc.sync.dma_start(ones_t[:], ones_ext[:])
            nc.sync.dma_start(mask_t[:], mask_ext[:])
            nc.sync.dma_start(ident_t[:], id_ext[:])

            # qkvT tiles: [q01 q23 k01 k23 v01 v23], each [128, N]
            qkvT = [pp.tile([128, N], F32R, tag=f"qkvT{i}", name=f"qkvT{i}")
                    for i in range(6)]
            a_bc = pp.tile([128, N], F32R, tag="a_bc")      # rs[n] broadcast
            # DVE/ACT lanes are partition-locked, so every derived row vector
            # lives at the partition where its consumer-matmul wants it.
            # (K=1 f32r matmuls only work at base partition 0 or 64, not 32.)
            #   rows p0 cols 0:N  = b2 (-mean)  [aug matmul A, uv row u @ p0]
            #   rows p0 cols N:2N = std         [aug matmul B, uv row vb @ p0]
            #   rows p64 cols 0:N = rs          [K=1 broadcast, ones row @ p64]
            rows = sp.tile([128, 2 * N], F32R, tag="rows")
            scr = sp.tile([128, N + 1024], F32, tag="scr")  # per-partition scratch

            # ---------------- phase A: stats + qkv projection ----------------
            with (
                tc.tile_pool(name="pA", bufs=1) as pa,
                tc.tile_pool(name="pAx", bufs=2) as pax,
                tc.tile_pool(name="psA", bufs=2, space="PSUM") as psa,
            ):
                xT = pa.tile([128, NK, N], F32R, tag="xT")
                wq = pa.tile([128, NK, 3 * CD], F32R, tag="wq")
                uv_t = pa.tile([1, 2 * 3 * CD], F32R, tag="uv")
                nc.sync.dma_start(uv_t[0:1, 0:3 * CD], uv_ext[0:1, :])
                nc.sync.dma_start(uv_t[0:1, 3 * CD:], uv_ext[1:2, :])
                xT_d = xT_ext[:].rearrange("(c p) n -> p c n", p=128)
                w_d = w_ext[:].rearrange("(c p) m -> p c m", p=128)
                for k in range(NK):
                    nc.sync.dma_start(xT[:, k, :], xT_d[:, k, :])
                    nc.sync.dma_start(wq[:, k, :], w_d[:, k, :])

                # stats: per 512-col block, colsums of x and x^2 replicated
                # to all 128 partitions (lhsT = all-ones [128,128])
                def _std_chain(p, ps_s, ps_q, dst_std):
                    sl = lambda c: scr[p:p + 1, c * 512:(c + 1) * 512]
                    mean, msq, m2n, var = sl(0), sl(1), sl(2), sl(3)
                    nc.vector.tensor_scalar(mean, ps_s[p:p + 1, :], 1.0 / DIM,
                                            None, op0=MUL)
                    nc.vector.tensor_scalar(msq, ps_q[p:p + 1, :], 1.0 / DIM,
                                            EPS, op0=MUL, op1=ADD)
                    nc.vector.tensor_tensor(m2n, mean, mean, op=MUL)
                    nc.vector.tensor_scalar(m2n, m2n, -1.0, None, op0=MUL)
                    nc.vector.tensor_tensor(var, msq, m2n, op=ADD)
                    nc.scalar.activation(dst_std, var, AF.Sqrt)
                    return mean

                for t in range(NT):
                    cs = slice(t * 512, (t + 1) * 512)
                    ps_s = psa.tile([128, 512], F32, tag="st_s")
                    ps_q = psa.tile([128, 512], F32, tag="st_q")
                    for k in range(NK):
                        xsq = pax.tile([128, 512], F32R, tag="xsq")
                        nc.scalar.activation(xsq[:], xT[:, k, cs], AF.Square)
                        nc.tensor.matmul(ps_s[:], ones_t[:], xT[:, k, cs],
                                         start=(k == 0), stop=(k == NK - 1),
                                         skip_group_check=True)
                        nc.tensor.matmul(ps_q[:], ones_t[:], xsq[:],
                                         start=(k == 0), stop=(k == NK - 1),
                                         skip_group_check=True)
                    # p0: std (cols N:2N) and b2 = -mean (cols 0:N)
                    mean0 = _std_chain(0, ps_s, ps_q,
                                       rows[0:1, N + t * 512:N + (t + 1) * 512])
                    nc.vector.tensor_scalar(rows[0:1, cs], mean0, -1.0, None,
                                            op0=MUL)
                    # p64: rs = 1/std
                    _std_chain(64, ps_s, ps_q, scr[64:65, 2048:2560].bitcast(F32R))
                    nc.vector.reciprocal(rows[64:65, cs],
                                         scr[64:65, 2048:2560].bitcast(F32R))
                    # broadcast rs to 128 partitions via K=1 matmul
                    ps_ab = psa.tile([128, 512], F32, tag="ab")
                    nc.tensor.matmul(ps_ab[:], ones_t[64:65, :],
                                     rows[64:65, cs], start=True, stop=True,
                                     skip_group_check=True)
                    nc.vector.tensor_copy(a_bc[:, cs], ps_ab[:])

                # qkv projection
                for ct in range(6):
                    ms = slice(ct * 128, (ct + 1) * 128)
                    for t in range(NT):
                        cs = slice(t * 512, (t + 1) * 512)
                        ps_m = psa.tile([128, 512], F32, tag="qkv")
                        for k in range(NK):
                            nc.tensor.matmul(ps_m[:], wq[:, k, ms],
                                             xT[:, k, cs], start=(k == 0),
                                             stop=False, skip_group_check=True)
                        nc.tensor.matmul(ps_m[:], uv_t[0:1, ms],
                                         rows[0:1, cs], start=False,
                                         stop=False, skip_group_check=True)
                        nc.tensor.matmul(
                            ps_m[:], uv_t[0:1, 3 * CD + ct * 128:3 * CD + (ct + 1) * 128],
                            rows[0:1, N + t * 512:N + (t + 1) * 512],
                            start=False, stop=True, skip_group_check=True)
                        nc.vector.tensor_tensor(qkvT[ct][:, cs], ps_m[:],
                                                a_bc[:, cs], op=MUL)

            # ---------------- phases B-D ----------------
            pbc_cm = tc.tile_pool(name="pBCD", bufs=1)
            pbc = pbc_cm.__enter__()
            v_nat = pbc.tile([128, NROW, HPC * VW], F32R, tag="v_nat")
            oT = [pbc.tile([128, N], F32R, tag=f"oT{i}", name=f"oT{i}")
                  for i in range(2)]

            # ---------------- phase B: v -> natural layout ----------------
            with tc.tile_pool(name="psB", bufs=2, space="PSUM") as psb:
                # ones columns (cols h*66+64 : h*66+66 of every row tile)
                for h in range(HPC):
                    dst = v_nat[:, :, h * VW + DH:h * VW + DH + 2]
                    src = ones_t[:, 0:2 * NROW].rearrange("p (a b) -> p a b", b=2)
                    nc.vector.tensor_scalar(dst, src, 0.0, 1.0, op0=MUL, op1=ADD)
                for hp in range(2):
                    for t in range(NROW):
                        ps_t = psb.tile([128, 128], F32R, tag="vt")
                        nc.tensor.transpose(
                            ps_t[:], qkvT[4 + hp][:, t * 128:(t + 1) * 128],
                            ident_t[:])
                        h0, h1 = 2 * hp, 2 * hp + 1
                        nc.vector.tensor_copy(
                            v_nat[:, t, h0 * VW:h0 * VW + DH], ps_t[:, 0:64])
                        nc.vector.tensor_copy(
                            v_nat[:, t, h1 * VW:h1 * VW + DH], ps_t[:, 64:128])

            # ---------------- phase C: attention per head ----------------
            with (
                tc.tile_pool(name="pC", bufs=3) as pc,
                tc.tile_pool(name="pCm", bufs=2) as pcm,
                tc.tile_pool(name="psC_s", bufs=3, space="PSUM") as psc_s,
                tc.tile_pool(name="psC_o", bufs=2, space="PSUM") as psc_o,
                tc.tile_pool(name="psC_r", bufs=2, space="PSUM") as psc_r,
            ):
                for h in range(HPC):
                    hp, off = h // 2, (h % 2) * 64
                    qT = qkvT[hp]
                    kT = qkvT[2 + hp]
                    for ib in range(NT):
                        i0 = ib * 512
                        isl = slice(i0, i0 + 512)
                        n_jt = 4 * (ib + 1)
                        o_ps = psc_o.tile([VW, 512], F32, tag="o")
                        for jt in range(n_jt):
                            j0 = jt * 128
                            s_ps = psc_s.tile([128, 512], F32, tag="s")
                            nc.tensor.matmul(
                                s_ps[:], kT[off:off + 64, j0:j0 + 128],
                                qT[off:off + 64, isl], start=True, stop=True,
                                skip_group_check=True)
                            if j0 < i0:  # fully causal tile
                                e_t = pc.tile([128, 512], F32R, tag="e")
                                nc.scalar.activation(e_t[:], s_ps[:], AF.Exp)
                                pv_rhs = e_t[:]
                            else:        # diagonal tile, offset o in {0,..,384}
                                o = j0 - i0
                                e_t = pc.tile([128, 512], F32R, tag="e")
                                nc.scalar.activation(e_t[:, o:512],
                                                     s_ps[:, o:512], AF.Exp)
                                e_m = pcm.tile([128, 512], F32R, tag="em")
                                if o > 0:
                                    nc.gpsimd.tensor_scalar(
                                        e_m[:, 0:o], a_bc[:, 0:o], 0.0, None,
                                        op0=MUL)
                                nc.gpsimd.tensor_tensor(
                                    e_m[:, o:o + 128], e_t[:, o:o + 128],
                                    mask_t[:], op=MUL)
                                if o < 384:
                                    nc.gpsimd.tensor_copy(e_m[:, o + 128:512],
                                                          e_t[:, o + 128:512])
                                pv_rhs = e_m[:]
                            nc.tensor.matmul(
                                o_ps[:], v_nat[:, jt, h * VW:(h + 1) * VW],
                                pv_rhs, start=(jt == 0), stop=(jt == n_jt - 1),
                                skip_group_check=True)
                        # normalize: rows 0:64 are sum(p*v), row 64 is sum(p)
                        rd = pc.tile([65, 512], F32R, tag="rd")
                        nc.vector.reciprocal(rd[64:65, :], o_ps[64:65, :])
                        rb_ps = psc_r.tile([64, 512], F32, tag="rb")
                        nc.tensor.matmul(rb_ps[:], ones_t[64:65, 0:64],
                                         rd[64:65, :], start=True, stop=True,
                                         skip_group_check=True)
                        rdb = pc.tile([64, 512], F32R, tag="rdb")
                        nc.vector.tensor_copy(rdb[:], rb_ps[:])
                        if off == 0:
                            nc.vector.tensor_tensor(oT[hp][0:64, isl],
                                                    o_ps[0:64, :], rdb[:],
                                                    op=MUL)
                        else:
                            # DVE lanes can't shift partitions; normalize at
                            # base 0 then DMA-shift to partitions 64:128
                            osh = pc.tile([64, 512], F32R, tag="osh")
                            nc.vector.tensor_tensor(osh[:], o_ps[0:64, :],
                                                    rdb[:], op=MUL)
                            nc.sync.dma_start(oT[hp][64:128, isl], osh[:])

            # ---------------- phase D: out projection ----------------
            with (
                tc.tile_pool(name="pD", bufs=3) as pd,
                tc.tile_pool(name="pDw", bufs=1) as pdw,
                tc.tile_pool(name="psD", bufs=3, space="PSUM") as psd,
            ):
                wo_t = pdw.tile([128, 2, DIM], F32R, tag="wo")
                wo_d = wo_ext[:].rearrange("(c p) m -> p c m", p=128)
                nc.sync.dma_start(wo_t[:, 0, :], wo_d[:, 0, :])
                nc.sync.dma_start(wo_t[:, 1, :], wo_d[:, 1, :])
                for t in range(NROW):
                    rsl = slice(t * 128, (t + 1) * 128)
                    for mt in range(2):
                        msl = slice(mt * 512, (mt + 1) * 512)
                        op_ps = psd.tile([128, 512], F32, tag="op")
                        nc.tensor.matmul(op_ps[:], oT[0][:, rsl],
                                         wo_t[:, 0, msl], start=True,
                                         stop=False, skip_group_check=True)
                        nc.tensor.matmul(op_ps[:], oT[1][:, rsl],
                                         wo_t[:, 1, msl], start=False,
                                         stop=True, skip_group_check=True)
                        ost = pd.tile([128, 512], F32, tag="ost")
                        nc.vector.tensor_copy(ost[:], op_ps[:])
                        nc.sync.dma_start(out_ext[rsl, msl], ost[:])
            pbc_cm.__exit__(None, None, None)

    nc.compile()
    return nc


_NC_CACHE = {}


def _get_nc():
    if "nc" not in _NC_CACHE:
        _NC_CACHE["nc"] = _build()
    return _NC_CACHE["nc"]


def _prep_in_maps(x, ln_w, ln_b, w_qkv, w_out):
    x = np.asarray(x, dtype=np.float32)
    ln_w = np.asarray(ln_w, dtype=np.float32)
    ln_b = np.asarray(ln_b, dtype=np.float32)
    w_qkv = np.asarray(w_qkv, dtype=np.float32)
    w_out = np.asarray(w_out, dtype=np.float32)

    ones = np.ones((128, 128), dtype=np.float32)
    # mask[jp, ii] = 1 iff jp <= ii  (keep j <= i)
    mask = np.triu(np.ones((128, 128), dtype=np.float32))
    ident = np.eye(128, dtype=np.float32)

    xTs = [_r32(x[b].T) for b in range(B)]

    in_maps = []
    for core in range(8):
        b, hg = core // 4, core % 4
        csl = slice(hg * CD, (hg + 1) * CD)
        # raw slices with SCALE folded into q
        w0 = np.concatenate([w_qkv[:, csl] * SCALE,
                             w_qkv[:, DIM + hg * CD:DIM + (hg + 1) * CD],
                             w_qkv[:, 2 * DIM + hg * CD:2 * DIM + (hg + 1) * CD]],
                            axis=1)
        wf = ln_w[:, None] * w0                      # ln_w folded
        u = wf.sum(axis=0)                           # pairs with -mean
        vb = ln_b @ w0                               # pairs with std (ln bias)
        uv = np.stack([u, vb]).astype(np.float32)
        in_maps.append({
            "xT": xTs[b],
            "wqkv": _r32(wf),
            "uv": _r32(uv),
            "wout": _r32(w_out[csl, :]),
            "ones": ones,
            "mask": mask,
            "ident": ident,
        })
    return in_maps


def _combine(results):
    out = np.empty((B, N, DIM), dtype=np.float32)
    for b in range(B):
        acc = results[b * 4]["out"].astype(np.float32)
        for hg in range(1, 4):
            acc = acc + results[b * 4 + hg]["out"]
        out[b] = acc
    return out


def kernel(x, ln_w, ln_b, w_qkv, w_out):
    nc = _get_nc()
    in_maps = _prep_in_maps(x, ln_w, ln_b, w_qkv, w_out)
    res = run_bass_kernel_spmd(nc, in_maps, core_ids=list(range(8)))
    return _combine(res.results)


def run_traced(x, ln_w, ln_b, w_qkv, w_out, **kwargs):
    """Run with NTFF profiling; returns (output, BassKernelResults)."""
    nc = _get_nc()
    in_maps = _prep_in_maps(x, ln_w, ln_b, w_qkv, w_out)
    res = run_bass_kernel_spmd(nc, in_maps, core_ids=list(range(8)),
                               trace=True, **kwargs)
    return _combine(res.results), res


# revision 19
# speedup vs baseline: 1.4067x; 1.4067x over previous
"""Fused LayerNorm + causal multi-head attention for Trainium2, 8 NeuronCores.

Problem: x[2,2048,1024] -> LN -> qkv proj (w_qkv[1024,3072]) -> 16-head causal
attention (d=64) -> out proj (w_out[1024,1024]).

Sharding (no cross-core communication):
  core c = b*4 + hg   (b in {0,1} batches, hg in {0..3} head-groups of 4 heads)
  Each core computes its batch's LN + its 4 heads' qkv/attention + a partial
  out-projection (its 256 rows of w_out). Host sums the 4 partials per batch.

Device algorithm (transposed layout: features on partitions, sequence on the
free axis; all matmuls in float32r = full-speed fp32 on the PE):
  1. stats: colsums of xT and xT^2 via ones-matmuls -> mean/std/rs rows
  2. LN folded into the QKV matmul: W premultiplied by ln_w (and SCALE for q)
     on host; the (-mean*u + std*vb) correction enters as 2 extra contraction
     rows; the rs[n] factor is one broadcast multiply on the psum epilogue
  3. qkvT[768,2048] tiles = W.T @ xT; v transposed to natural layout via PE
     transpose, with appended ones columns for the softmax denominator
  4. per head, per 512-wide i-block: sT[j,i] = kT.T@qT tiles, exp on ScalarE
     (no max subtraction: scores are O(6) for this distribution), causal
     masking assembled on GpSimd, PV + denominator accumulated as [66,512]
     psum, normalized by the reciprocal denominator broadcast via K=1 matmul
  5. partial out-proj [2048,1024] = oT.T @ w_out rows, DMA out
"""
import os
import sys

for _p in ("/opt/trn_rl_repo", "/root/.axon_site/_ro/trn_rl_repo"):
    if os.path.isdir(_p) and _p not in sys.path:
        sys.path.insert(0, _p)

import numpy as np

import concourse.bass as bass  # noqa: F401  (import side effects / debugging)
import concourse.mybir as mybir
import concourse.tile as tile
from concourse import bacc
from concourse.bass_utils import run_bass_kernel_spmd

F32 = mybir.dt.float32
F32R = mybir.dt.float32r
MUL = mybir.AluOpType.mult
ADD = mybir.AluOpType.add
AF = mybir.ActivationFunctionType

B, N, DIM = 2, 2048, 1024
HEADS, DH = 16, 64
HPC = 4            # heads per core
CD = HPC * DH      # 256 output channels per core
SCALE = DH ** -0.5
EPS = 1e-5
NT = N // 512      # 4 col-blocks of 512
NK = DIM // 128    # 8 contraction chunks
NROW = N // 128    # 16 row tiles of 128
VW = DH + 2        # 66: v columns + 2 ones columns per head


def _r32(x, bits=13):
    """tf32-style rounding: round-to-nearest, drop low `bits` mantissa bits."""
    v = np.ascontiguousarray(x, dtype=np.float32).view(np.uint32)
    v = (v + (1 << (bits - 1))) & np.uint32(~((1 << bits) - 1) & 0xFFFFFFFF)
    return v.view(np.float32)


def _build():
    nc = bacc.Bacc("TRN2", target_bir_lowering=False, debug=False)

    xT_ext = nc.declare_dram_parameter("xT", [DIM, N], F32R, isOutput=False)
    w_ext = nc.declare_dram_parameter("wqkv", [DIM, 3 * CD], F32R, isOutput=False)
    uv_ext = nc.declare_dram_parameter("uv", [2, 3 * CD], F32R, isOutput=False)
    wo_ext = nc.declare_dram_parameter("wout", [CD, DIM], F32R, isOutput=False)
    ones_ext = nc.declare_dram_parameter("ones", [128, 128], F32R, isOutput=False)
    mask_ext = nc.declare_dram_parameter("mask", [128, 128], F32R, isOutput=False)
    id_ext = nc.declare_dram_parameter("ident", [128, 128], F32R, isOutput=False)
    out_ext = nc.declare_dram_parameter("out", [N, DIM], F32, isOutput=True)

    with tile.TileContext(nc) as tc:
        with (
            nc.allow_low_precision(reason="float32r is 4-byte; psum stays f32"),
            tc.tile_pool(name="persist", bufs=1) as pp,
            tc.tile_pool(name="small", bufs=1) as sp,
        ):
            ones_t = pp.tile([128, 128], F32R, tag="ones")
            mask_t = pp.tile([128, 128], F32R, tag="mask")
            ident_t = pp.tile([128, 128], F32R, tag="ident")
            nc.sync.dma_start(ones_t[:], ones_ext[:])
            nc.sync.dma_start(mask_t[:], mask_ext[:])
            nc.sync.dma_start(ident_t[:], id_ext[:])

            # qkvT tiles: [q01 q23 k01 k23 v01 v23], each [128, N]
            qkvT = [pp.tile([128, N], F32R, tag=f"qkvT{i}", name=f"qkvT{i}")
                    for i in range(6)]
            a_bc = pp.tile([128, N], F32R, tag="a_bc")      # rs[n] broadcast
            # DVE/ACT lanes are partition-locked, so every derived row vector
            # lives at the partition where its consumer-matmul wants it.
            # (K=1 f32r matmuls only work at base partition 0 or 64, not 32.)
            #   rows p0 cols 0:N  = b2 (-mean)  [aug matmul A, uv row u @ p0]
            #   rows p0 cols N:2N = std         [aug matmul B, uv row vb @ p0]
            #   rows p64 cols 0:N = rs          [K=1 broadcast, ones row @ p64]
            rows = sp.tile([128, 2 * N], F32R, tag="rows")
            scr = sp.tile([128, N + 1024], F32, tag="scr")  # per-partition scratch

            # ---------------- phase A: stats + qkv projection ----------------
            with (
                tc.tile_pool(name="pA", bufs=1) as pa,
                tc.tile_pool(name="pAx", bufs=2) as pax,
                tc.tile_pool(name="psA", bufs=2, space="PSUM") as psa,
            ):
                xT = pa.tile([128, NK, N], F32R, tag="xT")
                wq = pa.tile([128, NK, 3 * CD], F32R, tag="wq")
                uv_t = pa.tile([1, 2 * 3 * CD], F32R, tag="uv")
                nc.sync.dma_start(uv_t[0:1, 0:3 * CD], uv_ext[0:1, :])
                nc.sync.dma_start(uv_t[0:1, 3 * CD:], uv_ext[1:2, :])
                xT_d = xT_ext[:].rearrange("(c p) n -> p c n", p=128)
                w_d = w_ext[:].rearrange("(c p) m -> p c m", p=128)
                for k in range(NK):
                    nc.sync.dma_start(xT[:, k, :], xT_d[:, k, :])
                    nc.sync.dma_start(wq[:, k, :], w_d[:, k, :])

                # stats: per 512-col block, colsums of x and x^2 replicated
                # to all 128 partitions (lhsT = all-ones [128,128])
                def _std_chain(p, ps_s, ps_q, dst_std):
                    sl = lambda c: scr[p:p + 1, c * 512:(c + 1) * 512]
                    mean, msq, m2n, var = sl(0), sl(1), sl(2), sl(3)
                    nc.vector.tensor_scalar(mean, ps_s[p:p + 1, :], 1.0 / DIM,
                                            None, op0=MUL)
                    nc.vector.tensor_scalar(msq, ps_q[p:p + 1, :], 1.0 / DIM,
                                            EPS, op0=MUL, op1=ADD)
                    nc.vector.tensor_tensor(m2n, mean, mean, op=MUL)
                    nc.vector.tensor_scalar(m2n, m2n, -1.0, None, op0=MUL)
                    nc.vector.tensor_tensor(var, msq, m2n, op=ADD)
                    nc.scalar.activation(dst_std, var, AF.Sqrt)
                    return mean

                for t in range(NT):
                    cs = slice(t * 512, (t + 1) * 512)
                    ps_s = psa.tile([128, 512], F32, tag="st_s")
                    ps_q = psa.tile([128, 512], F32, tag="st_q")
                    for k in range(NK):
                        xsq = pax.tile([128, 512], F32R, tag="xsq")
                        nc.scalar.activation(xsq[:], xT[:, k, cs], AF.Square)
                        nc.tensor.matmul(ps_s[:], ones_t[:], xT[:, k, cs],
                                         start=(k == 0), stop=(k == NK - 1),
                                         skip_group_check=True)
                        nc.tensor.matmul(ps_q[:], ones_t[:], xsq[:],
                                         start=(k == 0), stop=(k == NK - 1),
                                         skip_group_check=True)
                    # p0: std (cols N:2N) and b2 = -mean (cols 0:N)
                    mean0 = _std_chain(0, ps_s, ps_q,
                                       rows[0:1, N + t * 512:N + (t + 1) * 512])
                    nc.vector.tensor_scalar(rows[0:1, cs], mean0, -1.0, None,
                                            op0=MUL)
                    # p64: rs = 1/std
                    _std_chain(64, ps_s, ps_q, scr[64:65, 2048:2560].bitcast(F32R))
                    nc.vector.reciprocal(rows[64:65, cs],
                                         scr[64:65, 2048:2560].bitcast(F32R))
                    # broadcast rs to 128 partitions via K=1 matmul
                    ps_ab = psa.tile([128, 512], F32, tag="ab")
                    nc.tensor.matmul(ps_ab[:], ones_t[64:65, :],
                                     rows[64:65, cs], start=True, stop=True,
                                     skip_group_check=True)
                    nc.vector.tensor_copy(a_bc[:, cs], ps_ab[:])

                # qkv projection
                for ct in range(6):
                    ms = slice(ct * 128, (ct + 1) * 128)
                    for t in range(NT):
                        cs = slice(t * 512, (t + 1) * 512)
                        ps_m = psa.tile([128, 512], F32, tag="qkv")
                        for k in range(NK):
                            nc.tensor.matmul(ps_m[:], wq[:, k, ms],
                                             xT[:, k, cs], start=(k == 0),
                                             stop=False, skip_group_check=True)
                        nc.tensor.matmul(ps_m[:], uv_t[0:1, ms],
                                         rows[0:1, cs], start=False,
                                         stop=False, skip_group_check=True)
                        nc.tensor.matmul(
                            ps_m[:], uv_t[0:1, 3 * CD + ct * 128:3 * CD + (ct + 1) * 128],
                            rows[0:1, N + t * 512:N + (t + 1) * 512],
                            start=False, stop=True, skip_group_check=True)
                        nc.vector.tensor_tensor(qkvT[ct][:, cs], ps_m[:],
                                                a_bc[:, cs], op=MUL)

            # ---------------- phases B-D ----------------
            pbc_cm = tc.tile_pool(name="pBCD", bufs=1)
            pbc = pbc_cm.__enter__()
            v_nat = pbc.tile([128, NROW, HPC * VW], F32R, tag="v_nat")
            oT = [pbc.tile([128, N], F32R, tag=f"oT{i}", name=f"oT{i}")
                  for i in range(2)]

            # ---------------- phase B: v -> natural layout ----------------
            with tc.tile_pool(name="psB", bufs=2, space="PSUM") as psb:
                # ones columns (cols h*66+64 : h*66+66 of every row tile)
                for h in range(HPC):
                    dst = v_nat[:, :, h * VW + DH:h * VW + DH + 2]
                    src = ones_t[:, 0:2 * NROW].rearrange("p (a b) -> p a b", b=2)
                    nc.vector.tensor_scalar(dst, src, 0.0, 1.0, op0=MUL, op1=ADD)
                for hp in range(2):
                    for t in range(NROW):
                        ps_t = psb.tile([128, 128], F32R, tag="vt")
                        nc.tensor.transpose(
                            ps_t[:], qkvT[4 + hp][:, t * 128:(t + 1) * 128],
                            ident_t[:])
                        h0, h1 = 2 * hp, 2 * hp + 1
                        nc.vector.tensor_copy(
                            v_nat[:, t, h0 * VW:h0 * VW + DH], ps_t[:, 0:64])
                        nc.vector.tensor_copy(
                            v_nat[:, t, h1 * VW:h1 * VW + DH], ps_t[:, 64:128])

            # ---------------- phase C: attention per head ----------------
            with (
                tc.tile_pool(name="pC", bufs=3) as pc,
                tc.tile_pool(name="pCm", bufs=2) as pcm,
                tc.tile_pool(name="psC_s", bufs=3, space="PSUM") as psc_s,
                tc.tile_pool(name="psC_o", bufs=2, space="PSUM") as psc_o,
                tc.tile_pool(name="psC_r", bufs=2, space="PSUM") as psc_r,
            ):
                for h in range(HPC):
                    hp, off = h // 2, (h % 2) * 64
                    qT = qkvT[hp]
                    kT = qkvT[2 + hp]
                    for ib in range(NT):
                        i0 = ib * 512
                        isl = slice(i0, i0 + 512)
                        n_jt = 4 * (ib + 1)
                        o_ps = psc_o.tile([VW, 512], F32, tag="o")
                        for jt in range(n_jt):
                            j0 = jt * 128
                            s_ps = psc_s.tile([128, 512], F32, tag="s")
                            nc.tensor.matmul(
                                s_ps[:], kT[off:off + 64, j0:j0 + 128],
                                qT[off:off + 64, isl], start=True, stop=True,
                                skip_group_check=True)
                            vsl = v_nat[:, jt, h * VW:(h + 1) * VW]
                            last = (jt == n_jt - 1)
                            if j0 < i0:  # fully causal tile
                                e_t = pc.tile([128, 512], F32R, tag="e")
                                nc.scalar.activation(e_t[:], s_ps[:], AF.Exp)
                                nc.tensor.matmul(
                                    o_ps[:], vsl, e_t[:], start=(jt == 0),
                                    stop=last, skip_group_check=True)
                            else:        # diagonal tile, offset o in {0,..,384}
                                o = j0 - i0
                                e_t = pc.tile([128, 512], F32R, tag="e")
                                nc.scalar.activation(e_t[:, o:512],
                                                     s_ps[:, o:512], AF.Exp)
                                # mask only cols [o, o+128); psum cols < o are
                                # untouched by this tile (fully masked)
                                e_m = pcm.tile([128, 128], F32R, tag="em")
                                nc.gpsimd.tensor_tensor(
                                    e_m[:], e_t[:, o:o + 128], mask_t[:],
                                    op=MUL)
                                nc.tensor.matmul(
                                    o_ps[:, o:o + 128], vsl, e_m[:],
                                    start=(jt == 0), stop=last and o >= 384,
                                    skip_group_check=True)
                                if o < 384:
                                    nc.tensor.matmul(
                                        o_ps[:, o + 128:512], vsl,
                                        e_t[:, o + 128:512], start=False,
                                        stop=last, skip_group_check=True)
                        # normalize: rows 0:64 are sum(p*v), row 64 is sum(p).
                        # broadcast the denominator first, then one fast
                        # reciprocal on 64 lanes (vs 1-lane accurate recip)
                        dn = pc.tile([65, 512], F32R, tag="dn")
                        nc.scalar.activation(dn[64:65, :], o_ps[64:65, :],
                                             AF.Copy)
                        rb_ps = psc_r.tile([64, 512], F32, tag="rb")
                        nc.tensor.matmul(rb_ps[:], ones_t[64:65, 0:64],
                                         dn[64:65, :], start=True, stop=True,
                                         skip_group_check=True)
                        rdb = pc.tile([64, 512], F32, tag="rdb")
                        nc.vector.reciprocal_approx_fast(rdb[:], rb_ps[:])
                        if off == 0:
                            nc.vector.tensor_tensor(oT[hp][0:64, isl],
                                                    o_ps[0:64, :], rdb[:],
                                                    op=MUL)
                        else:
                            # DVE lanes can't shift partitions; normalize at
                            # base 0 then DMA-shift to partitions 64:128
                            osh = pc.tile([64, 512], F32R, tag="osh")
                            nc.vector.tensor_tensor(osh[:], o_ps[0:64, :],
                                                    rdb[:], op=MUL)
                            nc.sync.dma_start(oT[hp][64:128, isl], osh[:])

            # ---------------- phase D: out projection ----------------
            with (
                tc.tile_pool(name="pD", bufs=3) as pd,
                tc.tile_pool(name="pDw", bufs=1) as pdw,
                tc.tile_pool(name="psD", bufs=3, space="PSUM") as psd,
            ):
                wo_t = pdw.tile([128, 2, DIM], F32R, tag="wo")
                wo_d = wo_ext[:].rearrange("(c p) m -> p c m", p=128)
                nc.sync.dma_start(wo_t[:, 0, :], wo_d[:, 0, :])
                nc.sync.dma_start(wo_t[:, 1, :], wo_d[:, 1, :])
                for t in range(NROW):
                    rsl = slice(t * 128, (t + 1) * 128)
                    for mt in range(2):
                        msl = slice(mt * 512, (mt + 1) * 512)
                        op_ps = psd.tile([128, 512], F32, tag="op")
                        nc.tensor.matmul(op_ps[:], oT[0][:, rsl],
                                         wo_t[:, 0, msl], start=True,
                                         stop=False, skip_group_check=True)
                        nc.tensor.matmul(op_ps[:], oT[1][:, rsl],
                                         wo_t[:, 1, msl], start=False,
                                         stop=True, skip_group_check=True)
                        ost = pd.tile([128, 512], F32, tag="ost")
                        nc.vector.tensor_copy(ost[:], op_ps[:])
                        nc.sync.dma_start(out_ext[rsl, msl], ost[:])
            pbc_cm.__exit__(None, None, None)

    nc.compile()
    return nc


_NC_CACHE = {}


def _get_nc():
    if "nc" not in _NC_CACHE:
        _NC_CACHE["nc"] = _build()
    return _NC_CACHE["nc"]


def _prep_in_maps(x, ln_w, ln_b, w_qkv, w_out):
    x = np.asarray(x, dtype=np.float32)
    ln_w = np.asarray(ln_w, dtype=np.float32)
    ln_b = np.asarray(ln_b, dtype=np.float32)
    w_qkv = np.asarray(w_qkv, dtype=np.float32)
    w_out = np.asarray(w_out, dtype=np.float32)

    ones = np.ones((128, 128), dtype=np.float32)
    # mask[jp, ii] = 1 iff jp <= ii  (keep j <= i)
    mask = np.triu(np.ones((128, 128), dtype=np.float32))
    ident = np.eye(128, dtype=np.float32)

    xTs = [_r32(x[b].T) for b in range(B)]

    in_maps = []
    for core in range(8):
        b, hg = core // 4, core % 4
        csl = slice(hg * CD, (hg + 1) * CD)
        # raw slices with SCALE folded into q
        w0 = np.concatenate([w_qkv[:, csl] * SCALE,
                             w_qkv[:, DIM + hg * CD:DIM + (hg + 1) * CD],
                             w_qkv[:, 2 * DIM + hg * CD:2 * DIM + (hg + 1) * CD]],
                            axis=1)
        wf = ln_w[:, None] * w0                      # ln_w folded
        u = wf.sum(axis=0)                           # pairs with -mean
        vb = ln_b @ w0                               # pairs with std (ln bias)
        uv = np.stack([u, vb]).astype(np.float32)
        in_maps.append({
            "xT": xTs[b],
            "wqkv": _r32(wf),
            "uv": _r32(uv),
            "wout": _r32(w_out[csl, :]),
            "ones": ones,
            "mask": mask,
            "ident": ident,
        })
    return in_maps


def _combine(results):
    out = np.empty((B, N, DIM), dtype=np.float32)
    for b in range(B):
        acc = results[b * 4]["out"].astype(np.float32)
        for hg in range(1, 4):
            acc = acc + results[b * 4 + hg]["out"]
        out[b] = acc
    return out


def kernel(x, ln_w, ln_b, w_qkv, w_out):
    nc = _get_nc()
    in_maps = _prep_in_maps(x, ln_w, ln_b, w_qkv, w_out)
    res = run_bass_kernel_spmd(nc, in_maps, core_ids=list(range(8)))
    return _combine(res.results)


def run_traced(x, ln_w, ln_b, w_qkv, w_out, **kwargs):
    """Run with NTFF profiling; returns (output, BassKernelResults)."""
    nc = _get_nc()
    in_maps = _prep_in_maps(x, ln_w, ln_b, w_qkv, w_out)
    res = run_bass_kernel_spmd(nc, in_maps, core_ids=list(range(8)),
                               trace=True, **kwargs)
    return _combine(res.results), res


# revision 20
# speedup vs baseline: 1.5105x; 1.0738x over previous
"""Fused LayerNorm + causal multi-head attention for Trainium2, 8 NeuronCores.

Problem: x[2,2048,1024] -> LN -> qkv proj (w_qkv[1024,3072]) -> 16-head causal
attention (d=64) -> out proj (w_out[1024,1024]).

Sharding (no cross-core communication):
  core c = b*4 + hg   (b in {0,1} batches, hg in {0..3} head-groups of 4 heads)
  Each core computes its batch's LN + its 4 heads' qkv/attention + a partial
  out-projection (its 256 rows of w_out). Host sums the 4 partials per batch.

Device algorithm (transposed layout: features on partitions, sequence on the
free axis; all matmuls in float32r = full-speed fp32 on the PE):
  1. stats: colsums of xT and xT^2 via ones-matmuls -> mean/std/rs rows
  2. LN folded into the QKV matmul: W premultiplied by ln_w (and SCALE for q)
     on host; the (-mean*u + std*vb) correction enters as 2 extra contraction
     rows; the rs[n] factor is one broadcast multiply on the psum epilogue
  3. qkvT[768,2048] tiles = W.T @ xT; v transposed to natural layout via PE
     transpose, with appended ones columns for the softmax denominator
  4. per head, per 512-wide i-block: sT[j,i] = kT.T@qT tiles, exp on ScalarE
     (no max subtraction: scores are O(6) for this distribution), causal
     masking assembled on GpSimd, PV + denominator accumulated as [66,512]
     psum, normalized by the reciprocal denominator broadcast via K=1 matmul
  5. partial out-proj [2048,1024] = oT.T @ w_out rows, DMA out
"""
import os
import sys

for _p in ("/opt/trn_rl_repo", "/root/.axon_site/_ro/trn_rl_repo"):
    if os.path.isdir(_p) and _p not in sys.path:
        sys.path.insert(0, _p)

import numpy as np

import concourse.bass as bass  # noqa: F401  (import side effects / debugging)
import concourse.mybir as mybir
import concourse.tile as tile
from concourse import bacc
from concourse.bass_utils import run_bass_kernel_spmd

F32 = mybir.dt.float32
F32R = mybir.dt.float32r
BF16 = mybir.dt.bfloat16
MUL = mybir.AluOpType.mult
ADD = mybir.AluOpType.add
AF = mybir.ActivationFunctionType

B, N, DIM = 2, 2048, 1024
HEADS, DH = 16, 64
HPC = 4            # heads per core
CD = HPC * DH      # 256 output channels per core
SCALE = DH ** -0.5
EPS = 1e-5
NT = N // 512      # 4 col-blocks of 512
NK = DIM // 128    # 8 contraction chunks
NROW = N // 128    # 16 row tiles of 128
VW = DH + 2        # 66: v columns + 2 ones columns per head


def _r32(x, bits=13):
    """tf32-style rounding: round-to-nearest, drop low `bits` mantissa bits."""
    v = np.ascontiguousarray(x, dtype=np.float32).view(np.uint32)
    v = (v + (1 << (bits - 1))) & np.uint32(~((1 << bits) - 1) & 0xFFFFFFFF)
    return v.view(np.float32)


def _build():
    nc = bacc.Bacc("TRN2", target_bir_lowering=False, debug=False)

    xT_ext = nc.declare_dram_parameter("xT", [DIM, N], F32R, isOutput=False)
    w_ext = nc.declare_dram_parameter("wqkv", [DIM, 3 * CD], F32R, isOutput=False)
    uv_ext = nc.declare_dram_parameter("uv", [2, 3 * CD], F32R, isOutput=False)
    wo_ext = nc.declare_dram_parameter("wout", [CD, DIM], BF16, isOutput=False)
    ones_ext = nc.declare_dram_parameter("ones", [128, 128], F32R, isOutput=False)
    mask_ext = nc.declare_dram_parameter("mask", [128, 128], BF16, isOutput=False)
    id_ext = nc.declare_dram_parameter("ident", [128, 128], F32R, isOutput=False)
    out_ext = nc.declare_dram_parameter("out", [N, DIM], F32, isOutput=True)

    with tile.TileContext(nc) as tc:
        with (
            nc.allow_low_precision(reason="float32r is 4-byte; psum stays f32"),
            tc.tile_pool(name="persist", bufs=1) as pp,
            tc.tile_pool(name="small", bufs=1) as sp,
        ):
            ones_t = pp.tile([128, 128], F32R, tag="ones")
            mask_t = pp.tile([128, 128], BF16, tag="mask")
            ident_t = pp.tile([128, 128], F32R, tag="ident")
            nc.sync.dma_start(ones_t[:], ones_ext[:])
            nc.sync.dma_start(mask_t[:], mask_ext[:])
            nc.sync.dma_start(ident_t[:], id_ext[:])

            # qkvT tiles: [q01 q23 k01 k23 v01 v23], each [128, N]
            qkvT = [pp.tile([128, N], F32R, tag=f"qkvT{i}", name=f"qkvT{i}")
                    for i in range(6)]
            a_bc = pp.tile([128, N], F32R, tag="a_bc")      # rs[n] broadcast
            # DVE/ACT lanes are partition-locked, so every derived row vector
            # lives at the partition where its consumer-matmul wants it.
            # (K=1 f32r matmuls only work at base partition 0 or 64, not 32.)
            #   rows p0 cols 0:N  = b2 (-mean)  [aug matmul A, uv row u @ p0]
            #   rows p0 cols N:2N = std         [aug matmul B, uv row vb @ p0]
            #   rows p64 cols 0:N = rs          [K=1 broadcast, ones row @ p64]
            rows = sp.tile([128, 2 * N], F32R, tag="rows")
            scr = sp.tile([128, N + 1024], F32, tag="scr")  # per-partition scratch

            # ---------------- phase A: stats + qkv projection ----------------
            with (
                tc.tile_pool(name="pA", bufs=1) as pa,
                tc.tile_pool(name="pAx", bufs=2) as pax,
                tc.tile_pool(name="psA", bufs=2, space="PSUM") as psa,
            ):
                xT = pa.tile([128, NK, N], F32R, tag="xT")
                wq = pa.tile([128, NK, 3 * CD], F32R, tag="wq")
                uv_t = pa.tile([1, 2 * 3 * CD], F32R, tag="uv")
                nc.sync.dma_start(uv_t[0:1, 0:3 * CD], uv_ext[0:1, :])
                nc.sync.dma_start(uv_t[0:1, 3 * CD:], uv_ext[1:2, :])
                xT_d = xT_ext[:].rearrange("(c p) n -> p c n", p=128)
                w_d = w_ext[:].rearrange("(c p) m -> p c m", p=128)
                for k in range(NK):
                    nc.sync.dma_start(xT[:, k, :], xT_d[:, k, :])
                    nc.sync.dma_start(wq[:, k, :], w_d[:, k, :])

                # stats: per 512-col block, colsums of x and x^2 replicated
                # to all 128 partitions (lhsT = all-ones [128,128])
                def _std_chain(p, ps_s, ps_q, dst_std):
                    sl = lambda c: scr[p:p + 1, c * 512:(c + 1) * 512]
                    mean, msq, m2n, var = sl(0), sl(1), sl(2), sl(3)
                    nc.vector.tensor_scalar(mean, ps_s[p:p + 1, :], 1.0 / DIM,
                                            None, op0=MUL)
                    nc.vector.tensor_scalar(msq, ps_q[p:p + 1, :], 1.0 / DIM,
                                            EPS, op0=MUL, op1=ADD)
                    nc.vector.tensor_tensor(m2n, mean, mean, op=MUL)
                    nc.vector.tensor_scalar(m2n, m2n, -1.0, None, op0=MUL)
                    nc.vector.tensor_tensor(var, msq, m2n, op=ADD)
                    nc.scalar.activation(dst_std, var, AF.Sqrt)
                    return mean

                for t in range(NT):
                    cs = slice(t * 512, (t + 1) * 512)
                    ps_s = psa.tile([128, 512], F32, tag="st_s")
                    ps_q = psa.tile([128, 512], F32, tag="st_q")
                    for k in range(NK):
                        xsq = pax.tile([128, 512], F32R, tag="xsq")
                        nc.scalar.activation(xsq[:], xT[:, k, cs], AF.Square)
                        nc.tensor.matmul(ps_s[:], ones_t[:], xT[:, k, cs],
                                         start=(k == 0), stop=(k == NK - 1),
                                         skip_group_check=True)
                        nc.tensor.matmul(ps_q[:], ones_t[:], xsq[:],
                                         start=(k == 0), stop=(k == NK - 1),
                                         skip_group_check=True)
                    # p0: std (cols N:2N) and b2 = -mean (cols 0:N)
                    mean0 = _std_chain(0, ps_s, ps_q,
                                       rows[0:1, N + t * 512:N + (t + 1) * 512])
                    nc.vector.tensor_scalar(rows[0:1, cs], mean0, -1.0, None,
                                            op0=MUL)
                    # p64: rs = 1/std
                    _std_chain(64, ps_s, ps_q, scr[64:65, 2048:2560].bitcast(F32R))
                    nc.vector.reciprocal(rows[64:65, cs],
                                         scr[64:65, 2048:2560].bitcast(F32R))
                    # broadcast rs to 128 partitions via K=1 matmul
                    ps_ab = psa.tile([128, 512], F32, tag="ab")
                    nc.tensor.matmul(ps_ab[:], ones_t[64:65, :],
                                     rows[64:65, cs], start=True, stop=True,
                                     skip_group_check=True)
                    nc.vector.tensor_copy(a_bc[:, cs], ps_ab[:])

                # qkv projection
                for ct in range(6):
                    ms = slice(ct * 128, (ct + 1) * 128)
                    for t in range(NT):
                        cs = slice(t * 512, (t + 1) * 512)
                        ps_m = psa.tile([128, 512], F32, tag="qkv")
                        for k in range(NK):
                            nc.tensor.matmul(ps_m[:], wq[:, k, ms],
                                             xT[:, k, cs], start=(k == 0),
                                             stop=False, skip_group_check=True)
                        nc.tensor.matmul(ps_m[:], uv_t[0:1, ms],
                                         rows[0:1, cs], start=False,
                                         stop=False, skip_group_check=True)
                        nc.tensor.matmul(
                            ps_m[:], uv_t[0:1, 3 * CD + ct * 128:3 * CD + (ct + 1) * 128],
                            rows[0:1, N + t * 512:N + (t + 1) * 512],
                            start=False, stop=True, skip_group_check=True)
                        nc.vector.tensor_tensor(qkvT[ct][:, cs], ps_m[:],
                                                a_bc[:, cs], op=MUL)

            # ---------------- phases B-D ----------------
            pbc_cm = tc.tile_pool(name="pBCD", bufs=1)
            pbc = pbc_cm.__enter__()
            v_nat = pbc.tile([128, NROW, HPC * VW], BF16, tag="v_nat")
            oT = [pbc.tile([128, N], BF16, tag=f"oT{i}", name=f"oT{i}")
                  for i in range(2)]

            # ---------------- phase B: v -> natural layout ----------------
            with tc.tile_pool(name="psB", bufs=2, space="PSUM") as psb:
                # ones columns (cols h*66+64 : h*66+66 of every row tile)
                for h in range(HPC):
                    dst = v_nat[:, :, h * VW + DH:h * VW + DH + 2]
                    src = ones_t[:, 0:2 * NROW].rearrange("p (a b) -> p a b", b=2)
                    nc.vector.tensor_scalar(dst, src, 0.0, 1.0, op0=MUL, op1=ADD)
                for hp in range(2):
                    for t in range(NROW):
                        ps_t = psb.tile([128, 128], F32R, tag="vt")
                        nc.tensor.transpose(
                            ps_t[:], qkvT[4 + hp][:, t * 128:(t + 1) * 128],
                            ident_t[:])
                        h0, h1 = 2 * hp, 2 * hp + 1
                        nc.vector.tensor_copy(
                            v_nat[:, t, h0 * VW:h0 * VW + DH], ps_t[:, 0:64])
                        nc.vector.tensor_copy(
                            v_nat[:, t, h1 * VW:h1 * VW + DH], ps_t[:, 64:128])

            # ---------------- phase C: attention per head ----------------
            with (
                tc.tile_pool(name="pC", bufs=3) as pc,
                tc.tile_pool(name="pCm", bufs=2) as pcm,
                tc.tile_pool(name="psC_s", bufs=3, space="PSUM") as psc_s,
                tc.tile_pool(name="psC_o", bufs=2, space="PSUM") as psc_o,
                tc.tile_pool(name="psC_r", bufs=2, space="PSUM") as psc_r,
            ):
                for h in range(HPC):
                    hp, off = h // 2, (h % 2) * 64
                    qT = qkvT[hp]
                    kT = qkvT[2 + hp]
                    for ib in range(NT):
                        i0 = ib * 512
                        isl = slice(i0, i0 + 512)
                        n_jt = 4 * (ib + 1)
                        o_ps = psc_o.tile([VW, 512], F32, tag="o")
                        for jt in range(n_jt):
                            j0 = jt * 128
                            so = max(0, j0 - i0)
                            s_ps = psc_s.tile([128, 512], F32, tag="s")
                            nc.tensor.matmul(
                                s_ps[:, so:512], kT[off:off + 64, j0:j0 + 128],
                                qT[off:off + 64, i0 + so:i0 + 512],
                                start=True, stop=True, skip_group_check=True)
                            vsl = v_nat[:, jt, h * VW:(h + 1) * VW]
                            last = (jt == n_jt - 1)
                            if j0 < i0:  # fully causal tile
                                e_t = pc.tile([128, 512], BF16, tag="e")
                                nc.scalar.activation(e_t[:], s_ps[:], AF.Exp)
                                nc.tensor.matmul(
                                    o_ps[:], vsl, e_t[:], start=(jt == 0),
                                    stop=last, skip_group_check=True)
                            else:        # diagonal tile, offset o in {0,..,384}
                                o = j0 - i0
                                e_t = pc.tile([128, 512], BF16, tag="e")
                                nc.scalar.activation(e_t[:, o:512],
                                                     s_ps[:, o:512], AF.Exp)
                                # mask only cols [o, o+128); psum cols < o are
                                # untouched by this tile (fully masked)
                                e_m = pcm.tile([128, 128], BF16, tag="em")
                                nc.gpsimd.tensor_tensor(
                                    e_m[:], e_t[:, o:o + 128], mask_t[:],
                                    op=MUL)
                                nc.tensor.matmul(
                                    o_ps[:, o:o + 128], vsl, e_m[:],
                                    start=(jt == 0), stop=last and o >= 384,
                                    skip_group_check=True)
                                if o < 384:
                                    nc.tensor.matmul(
                                        o_ps[:, o + 128:512], vsl,
                                        e_t[:, o + 128:512], start=False,
                                        stop=last, skip_group_check=True)
                        # normalize: rows 0:64 are sum(p*v), row 64 is sum(p).
                        # broadcast the denominator first, then one fast
                        # reciprocal on 64 lanes (vs 1-lane accurate recip)
                        dn = pc.tile([65, 512], F32R, tag="dn")
                        nc.scalar.activation(dn[64:65, :], o_ps[64:65, :],
                                             AF.Copy)
                        rb_ps = psc_r.tile([64, 512], F32, tag="rb")
                        nc.tensor.matmul(rb_ps[:], ones_t[64:65, 0:64],
                                         dn[64:65, :], start=True, stop=True,
                                         skip_group_check=True)
                        rdb = pc.tile([64, 512], F32, tag="rdb")
                        nc.vector.reciprocal_approx_fast(rdb[:], rb_ps[:])
                        if off == 0:
                            nc.vector.tensor_tensor(oT[hp][0:64, isl],
                                                    o_ps[0:64, :], rdb[:],
                                                    op=MUL)
                        else:
                            # DVE lanes can't shift partitions; normalize at
                            # base 0 then DMA-shift to partitions 64:128
                            osh = pc.tile([64, 512], BF16, tag="osh")
                            nc.vector.tensor_tensor(osh[:], o_ps[0:64, :],
                                                    rdb[:], op=MUL)
                            nc.sync.dma_start(oT[hp][64:128, isl], osh[:])

            # ---------------- phase D: out projection ----------------
            with (
                tc.tile_pool(name="pD", bufs=3) as pd,
                tc.tile_pool(name="pDw", bufs=1) as pdw,
                tc.tile_pool(name="psD", bufs=3, space="PSUM") as psd,
            ):
                wo_t = pdw.tile([128, 2, DIM], BF16, tag="wo")
                wo_d = wo_ext[:].rearrange("(c p) m -> p c m", p=128)
                nc.sync.dma_start(wo_t[:, 0, :], wo_d[:, 0, :])
                nc.sync.dma_start(wo_t[:, 1, :], wo_d[:, 1, :])
                for t in range(NROW):
                    rsl = slice(t * 128, (t + 1) * 128)
                    for mt in range(2):
                        msl = slice(mt * 512, (mt + 1) * 512)
                        op_ps = psd.tile([128, 512], F32, tag="op")
                        nc.tensor.matmul(op_ps[:], oT[0][:, rsl],
                                         wo_t[:, 0, msl], start=True,
                                         stop=False, skip_group_check=True)
                        nc.tensor.matmul(op_ps[:], oT[1][:, rsl],
                                         wo_t[:, 1, msl], start=False,
                                         stop=True, skip_group_check=True)
                        ost = pd.tile([128, 512], F32, tag="ost")
                        nc.vector.tensor_copy(ost[:], op_ps[:])
                        nc.sync.dma_start(out_ext[rsl, msl], ost[:])
            pbc_cm.__exit__(None, None, None)

    nc.compile()
    return nc


_NC_CACHE = {}


def _get_nc():
    if "nc" not in _NC_CACHE:
        _NC_CACHE["nc"] = _build()
    return _NC_CACHE["nc"]


def _prep_in_maps(x, ln_w, ln_b, w_qkv, w_out):
    x = np.asarray(x, dtype=np.float32)
    ln_w = np.asarray(ln_w, dtype=np.float32)
    ln_b = np.asarray(ln_b, dtype=np.float32)
    w_qkv = np.asarray(w_qkv, dtype=np.float32)
    w_out = np.asarray(w_out, dtype=np.float32)

    import ml_dtypes
    ones = np.ones((128, 128), dtype=np.float32)
    # mask[jp, ii] = 1 iff jp <= ii  (keep j <= i)
    mask = np.triu(np.ones((128, 128), dtype=ml_dtypes.bfloat16))
    ident = np.eye(128, dtype=np.float32)

    xTs = [_r32(x[b].T) for b in range(B)]

    in_maps = []
    for core in range(8):
        b, hg = core // 4, core % 4
        csl = slice(hg * CD, (hg + 1) * CD)
        # raw slices with SCALE folded into q
        w0 = np.concatenate([w_qkv[:, csl] * SCALE,
                             w_qkv[:, DIM + hg * CD:DIM + (hg + 1) * CD],
                             w_qkv[:, 2 * DIM + hg * CD:2 * DIM + (hg + 1) * CD]],
                            axis=1)
        wf = ln_w[:, None] * w0                      # ln_w folded
        u = wf.sum(axis=0)                           # pairs with -mean
        vb = ln_b @ w0                               # pairs with std (ln bias)
        uv = np.stack([u, vb]).astype(np.float32)
        in_maps.append({
            "xT": xTs[b],
            "wqkv": _r32(wf),
            "uv": _r32(uv),
            "wout": w_out[csl, :].astype(ml_dtypes.bfloat16),
            "ones": ones,
            "mask": mask,
            "ident": ident,
        })
    return in_maps


def _combine(results):
    out = np.empty((B, N, DIM), dtype=np.float32)
    for b in range(B):
        acc = results[b * 4]["out"].astype(np.float32)
        for hg in range(1, 4):
            acc = acc + results[b * 4 + hg]["out"]
        out[b] = acc
    return out


def kernel(x, ln_w, ln_b, w_qkv, w_out):
    nc = _get_nc()
    in_maps = _prep_in_maps(x, ln_w, ln_b, w_qkv, w_out)
    res = run_bass_kernel_spmd(nc, in_maps, core_ids=list(range(8)))
    return _combine(res.results)


def run_traced(x, ln_w, ln_b, w_qkv, w_out, **kwargs):
    """Run with NTFF profiling; returns (output, BassKernelResults)."""
    nc = _get_nc()
    in_maps = _prep_in_maps(x, ln_w, ln_b, w_qkv, w_out)
    res = run_bass_kernel_spmd(nc, in_maps, core_ids=list(range(8)),
                               trace=True, **kwargs)
    return _combine(res.results), res


# revision 22
# speedup vs baseline: 1.7170x; 1.1367x over previous
"""Fused LayerNorm + causal multi-head attention for Trainium2, 8 NeuronCores.

Problem: x[2,2048,1024] -> LN -> qkv proj (w_qkv[1024,3072]) -> 16-head causal
attention (d=64) -> out proj (w_out[1024,1024]).

Sharding (no cross-core communication):
  core c = b*4 + hg   (b in {0,1} batches, hg in {0..3} head-groups of 4 heads)
  Each core computes its batch's LN + its 4 heads' qkv/attention + a partial
  out-projection (its 256 rows of w_out). Host sums the 4 partials per batch.

Device algorithm (transposed layout: features on partitions, sequence on the
free axis; all matmuls in float32r = full-speed fp32 on the PE):
  1. stats: colsums of xT and xT^2 via ones-matmuls -> mean/std/rs rows
  2. LN folded into the QKV matmul: W premultiplied by ln_w (and SCALE for q)
     on host; the (-mean*u + std*vb) correction enters as 2 extra contraction
     rows; the rs[n] factor is one broadcast multiply on the psum epilogue
  3. qkvT[768,2048] tiles = W.T @ xT; v transposed to natural layout via PE
     transpose, with appended ones columns for the softmax denominator
  4. per head, per 512-wide i-block: sT[j,i] = kT.T@qT tiles, exp on ScalarE
     (no max subtraction: scores are O(6) for this distribution), causal
     masking assembled on GpSimd, PV + denominator accumulated as [66,512]
     psum, normalized by the reciprocal denominator broadcast via K=1 matmul
  5. partial out-proj [2048,1024] = oT.T @ w_out rows, DMA out
"""
import os
import sys

for _p in ("/opt/trn_rl_repo", "/root/.axon_site/_ro/trn_rl_repo"):
    if os.path.isdir(_p) and _p not in sys.path:
        sys.path.insert(0, _p)

import numpy as np

import concourse.bass as bass  # noqa: F401  (import side effects / debugging)
import concourse.mybir as mybir
import concourse.tile as tile
from concourse import bacc
from concourse.bass_utils import run_bass_kernel_spmd

F32 = mybir.dt.float32
F32R = mybir.dt.float32r
BF16 = mybir.dt.bfloat16
MUL = mybir.AluOpType.mult
ADD = mybir.AluOpType.add
AF = mybir.ActivationFunctionType

B, N, DIM = 2, 2048, 1024
HEADS, DH = 16, 64
HPC = 4            # heads per core
CD = HPC * DH      # 256 output channels per core
SCALE = DH ** -0.5
EPS = 1e-5
NT = N // 512      # 4 col-blocks of 512
NK = DIM // 128    # 8 contraction chunks
NROW = N // 128    # 16 row tiles of 128
VW = DH + 2        # 66: v columns + 2 ones columns per head


def _r32(x, bits=13):
    """tf32-style rounding: round-to-nearest, drop low `bits` mantissa bits."""
    v = np.ascontiguousarray(x, dtype=np.float32).view(np.uint32)
    v = (v + (1 << (bits - 1))) & np.uint32(~((1 << bits) - 1) & 0xFFFFFFFF)
    return v.view(np.float32)


def _build():
    nc = bacc.Bacc("TRN2", target_bir_lowering=False, debug=False)

    xT_ext = nc.declare_dram_parameter("xT", [DIM, N], BF16, isOutput=False)
    w_ext = nc.declare_dram_parameter("wqkv", [DIM, 3 * CD], BF16, isOutput=False)
    uv_ext = nc.declare_dram_parameter("uv", [2, 3 * CD], F32R, isOutput=False)
    wo_ext = nc.declare_dram_parameter("wout", [CD, DIM], BF16, isOutput=False)
    ones_ext = nc.declare_dram_parameter("ones", [128, 128], F32R, isOutput=False)
    mask_ext = nc.declare_dram_parameter("mask", [128, 128], BF16, isOutput=False)
    id_ext = nc.declare_dram_parameter("ident", [128, 128], BF16, isOutput=False)
    out_ext = nc.declare_dram_parameter("out", [N, DIM], F32, isOutput=True)

    with tile.TileContext(nc) as tc:
        with (
            nc.allow_low_precision(reason="float32r is 4-byte; psum stays f32"),
            tc.tile_pool(name="persist", bufs=1) as pp,
            tc.tile_pool(name="small", bufs=1) as sp,
        ):
            ones_t = pp.tile([128, 128], F32R, tag="ones")
            mask_t = pp.tile([128, 128], BF16, tag="mask")
            ident_t = pp.tile([128, 128], BF16, tag="ident")
            nc.sync.dma_start(ones_t[:], ones_ext[:])
            nc.sync.dma_start(mask_t[:], mask_ext[:])
            nc.sync.dma_start(ident_t[:], id_ext[:])
            ones_b = pp.tile([128, 128], BF16, tag="ones_b")
            nc.vector.tensor_scalar(ones_b[:], ones_t[:], 0.0, 1.0,
                                    op0=MUL, op1=ADD)

            # qkvT tiles: [q01 q23 k01 k23 v01 v23], each [128, N]
            qkvT = [pp.tile([128, N], BF16, tag=f"qkvT{i}", name=f"qkvT{i}")
                    for i in range(6)]
            a_bc = pp.tile([128, N], F32R, tag="a_bc")      # rs[n] broadcast
            # DVE/ACT lanes are partition-locked, so every derived row vector
            # lives at the partition where its consumer-matmul wants it.
            # (K=1 f32r matmuls only work at base partition 0 or 64, not 32.)
            #   rows p0 cols 0:N  = b2 (-mean)  [aug matmul A, uv row u @ p0]
            #   rows p0 cols N:2N = std         [aug matmul B, uv row vb @ p0]
            #   rows p64 cols 0:N = rs          [K=1 broadcast, ones row @ p64]
            rows = sp.tile([128, 2 * N], F32R, tag="rows")
            scr = sp.tile([128, N + 1024], F32, tag="scr")  # per-partition scratch

            # ---------------- phase A: stats + qkv projection ----------------
            with (
                tc.tile_pool(name="pA", bufs=1) as pa,
                tc.tile_pool(name="pAx", bufs=2) as pax,
                tc.tile_pool(name="psA", bufs=2, space="PSUM") as psa,
            ):
                xT = pa.tile([128, NK, N], BF16, tag="xT")
                wq = pa.tile([128, NK, 3 * CD], BF16, tag="wq")
                uv_t = pa.tile([1, 2 * 3 * CD], F32R, tag="uv")
                nc.sync.dma_start(uv_t[0:1, 0:3 * CD], uv_ext[0:1, :])
                nc.sync.dma_start(uv_t[0:1, 3 * CD:], uv_ext[1:2, :])
                xT_d = xT_ext[:].rearrange("(c p) n -> p c n", p=128)
                w_d = w_ext[:].rearrange("(c p) m -> p c m", p=128)
                for k in range(NK):
                    nc.sync.dma_start(xT[:, k, :], xT_d[:, k, :])
                    nc.sync.dma_start(wq[:, k, :], w_d[:, k, :])

                # stats: per 512-col block, colsums of x and x^2 replicated
                # to all 128 partitions (lhsT = all-ones [128,128])
                def _std_chain(p, ps_s, ps_q, dst_std):
                    sl = lambda c: scr[p:p + 1, c * 512:(c + 1) * 512]
                    mean, msq, m2n, var = sl(0), sl(1), sl(2), sl(3)
                    nc.vector.tensor_scalar(mean, ps_s[p:p + 1, :], 1.0 / DIM,
                                            None, op0=MUL)
                    nc.vector.tensor_scalar(msq, ps_q[p:p + 1, :], 1.0 / DIM,
                                            EPS, op0=MUL, op1=ADD)
                    nc.vector.tensor_tensor(m2n, mean, mean, op=MUL)
                    nc.vector.tensor_scalar(m2n, m2n, -1.0, None, op0=MUL)
                    nc.vector.tensor_tensor(var, msq, m2n, op=ADD)
                    nc.scalar.activation(dst_std, var, AF.Sqrt)
                    return mean

                for t in range(NT):
                    cs = slice(t * 512, (t + 1) * 512)
                    ps_s = psa.tile([128, 512], F32, tag="st_s")
                    ps_q = psa.tile([128, 512], F32, tag="st_q")
                    for k in range(NK):
                        xsq = pax.tile([128, 512], BF16, tag="xsq")
                        nc.scalar.activation(xsq[:], xT[:, k, cs], AF.Square)
                        nc.tensor.matmul(ps_s[:], ones_b[:], xT[:, k, cs],
                                         start=(k == 0), stop=(k == NK - 1),
                                         skip_group_check=True)
                        nc.tensor.matmul(ps_q[:], ones_b[:], xsq[:],
                                         start=(k == 0), stop=(k == NK - 1),
                                         skip_group_check=True)
                    # p0: std (cols N:2N) and b2 = -mean (cols 0:N)
                    mean0 = _std_chain(0, ps_s, ps_q,
                                       rows[0:1, N + t * 512:N + (t + 1) * 512])
                    nc.vector.tensor_scalar(rows[0:1, cs], mean0, -1.0, None,
                                            op0=MUL)
                    # p64: rs = 1/std
                    _std_chain(64, ps_s, ps_q, scr[64:65, 2048:2560].bitcast(F32R))
                    nc.vector.reciprocal(rows[64:65, cs],
                                         scr[64:65, 2048:2560].bitcast(F32R))
                    # broadcast rs to 128 partitions via K=1 matmul
                    ps_ab = psa.tile([128, 512], F32, tag="ab")
                    nc.tensor.matmul(ps_ab[:], ones_t[64:65, :],
                                     rows[64:65, cs], start=True, stop=True,
                                     skip_group_check=True)
                    nc.vector.tensor_copy(a_bc[:, cs], ps_ab[:])

                # qkv projection
                for ct in range(6):
                    ms = slice(ct * 128, (ct + 1) * 128)
                    for t in range(NT):
                        cs = slice(t * 512, (t + 1) * 512)
                        ps_m = psa.tile([128, 512], F32, tag="qkv")
                        for k in range(NK):
                            nc.tensor.matmul(ps_m[:], wq[:, k, ms],
                                             xT[:, k, cs], start=(k == 0),
                                             stop=False, skip_group_check=True)
                        nc.tensor.matmul(ps_m[:], uv_t[0:1, ms],
                                         rows[0:1, cs], start=False,
                                         stop=False, skip_group_check=True)
                        nc.tensor.matmul(
                            ps_m[:], uv_t[0:1, 3 * CD + ct * 128:3 * CD + (ct + 1) * 128],
                            rows[0:1, N + t * 512:N + (t + 1) * 512],
                            start=False, stop=True, skip_group_check=True)
                        nc.vector.tensor_tensor(qkvT[ct][:, cs], ps_m[:],
                                                a_bc[:, cs], op=MUL)

            # ---------------- phases B-D ----------------
            pbc_cm = tc.tile_pool(name="pBCD", bufs=1)
            pbc = pbc_cm.__enter__()
            v_nat = pbc.tile([128, NROW, HPC * VW], BF16, tag="v_nat")
            oT = [pbc.tile([128, N], BF16, tag=f"oT{i}", name=f"oT{i}")
                  for i in range(2)]

            # ---------------- phase B: v -> natural layout ----------------
            with tc.tile_pool(name="psB", bufs=2, space="PSUM") as psb:
                # ones columns (cols h*66+64 : h*66+66 of every row tile)
                for h in range(HPC):
                    dst = v_nat[:, :, h * VW + DH:h * VW + DH + 2]
                    src = ones_t[:, 0:2 * NROW].rearrange("p (a b) -> p a b", b=2)
                    nc.vector.tensor_scalar(dst, src, 0.0, 1.0, op0=MUL, op1=ADD)
                for hp in range(2):
                    for t in range(NROW):
                        ps_t = psb.tile([128, 128], BF16, tag="vt")
                        nc.tensor.transpose(
                            ps_t[:], qkvT[4 + hp][:, t * 128:(t + 1) * 128],
                            ident_t[:])
                        h0, h1 = 2 * hp, 2 * hp + 1
                        nc.vector.tensor_copy(
                            v_nat[:, t, h0 * VW:h0 * VW + DH], ps_t[:, 0:64])
                        nc.vector.tensor_copy(
                            v_nat[:, t, h1 * VW:h1 * VW + DH], ps_t[:, 64:128])

            # ---------------- phase C: attention per head ----------------
            with (
                tc.tile_pool(name="pC", bufs=3) as pc,
                tc.tile_pool(name="pCm", bufs=2) as pcm,
                tc.tile_pool(name="psC_s", bufs=3, space="PSUM") as psc_s,
                tc.tile_pool(name="psC_o", bufs=2, space="PSUM") as psc_o,
                tc.tile_pool(name="psC_r", bufs=2, space="PSUM") as psc_r,
            ):
                for h in range(HPC):
                    hp, off = h // 2, (h % 2) * 64
                    qT = qkvT[hp]
                    kT = qkvT[2 + hp]
                    for ib in range(NT):
                        i0 = ib * 512
                        isl = slice(i0, i0 + 512)
                        n_jt = 4 * (ib + 1)
                        o_ps = psc_o.tile([VW, 512], F32, tag="o")
                        for jt in range(n_jt):
                            j0 = jt * 128
                            so = max(0, j0 - i0)
                            s_ps = psc_s.tile([128, 512], F32, tag="s")
                            nc.tensor.matmul(
                                s_ps[:, so:512], kT[off:off + 64, j0:j0 + 128],
                                qT[off:off + 64, i0 + so:i0 + 512],
                                start=True, stop=True, skip_group_check=True)
                            vsl = v_nat[:, jt, h * VW:(h + 1) * VW]
                            last = (jt == n_jt - 1)
                            if j0 < i0:  # fully causal tile
                                e_t = pc.tile([128, 512], BF16, tag="e")
                                nc.scalar.activation(e_t[:], s_ps[:], AF.Exp)
                                nc.tensor.matmul(
                                    o_ps[:], vsl, e_t[:], start=(jt == 0),
                                    stop=last, skip_group_check=True)
                            else:        # diagonal tile, offset o in {0,..,384}
                                o = j0 - i0
                                e_t = pc.tile([128, 512], BF16, tag="e")
                                nc.scalar.activation(e_t[:, o:512],
                                                     s_ps[:, o:512], AF.Exp)
                                # mask only cols [o, o+128); psum cols < o are
                                # untouched by this tile (fully masked)
                                e_m = pcm.tile([128, 128], BF16, tag="em")
                                nc.gpsimd.tensor_tensor(
                                    e_m[:], e_t[:, o:o + 128], mask_t[:],
                                    op=MUL)
                                nc.tensor.matmul(
                                    o_ps[:, o:o + 128], vsl, e_m[:],
                                    start=(jt == 0), stop=last and o >= 384,
                                    skip_group_check=True)
                                if o < 384:
                                    nc.tensor.matmul(
                                        o_ps[:, o + 128:512], vsl,
                                        e_t[:, o + 128:512], start=False,
                                        stop=last, skip_group_check=True)
                        # normalize: rows 0:64 are sum(p*v), row 64 is sum(p).
                        # broadcast the denominator first, then one fast
                        # reciprocal on 64 lanes (vs 1-lane accurate recip)
                        dn = pc.tile([65, 512], F32R, tag="dn")
                        nc.scalar.activation(dn[64:65, :], o_ps[64:65, :],
                                             AF.Copy)
                        rb_ps = psc_r.tile([64, 512], F32, tag="rb")
                        nc.tensor.matmul(rb_ps[:], ones_t[64:65, 0:64],
                                         dn[64:65, :], start=True, stop=True,
                                         skip_group_check=True)
                        rdb = pc.tile([64, 512], F32, tag="rdb")
                        nc.vector.reciprocal_approx_fast(rdb[:], rb_ps[:])
                        if off == 0:
                            nc.vector.tensor_tensor(oT[hp][0:64, isl],
                                                    o_ps[0:64, :], rdb[:],
                                                    op=MUL)
                        else:
                            # DVE lanes can't shift partitions; normalize at
                            # base 0 then DMA-shift to partitions 64:128
                            osh = pc.tile([64, 512], BF16, tag="osh")
                            nc.vector.tensor_tensor(osh[:], o_ps[0:64, :],
                                                    rdb[:], op=MUL)
                            nc.sync.dma_start(oT[hp][64:128, isl], osh[:])

            # ---------------- phase D: out projection ----------------
            with (
                tc.tile_pool(name="pD", bufs=3) as pd,
                tc.tile_pool(name="pDw", bufs=1) as pdw,
                tc.tile_pool(name="psD", bufs=3, space="PSUM") as psd,
            ):
                wo_t = pdw.tile([128, 2, DIM], BF16, tag="wo")
                wo_d = wo_ext[:].rearrange("(c p) m -> p c m", p=128)
                nc.sync.dma_start(wo_t[:, 0, :], wo_d[:, 0, :])
                nc.sync.dma_start(wo_t[:, 1, :], wo_d[:, 1, :])
                for t in range(NROW):
                    rsl = slice(t * 128, (t + 1) * 128)
                    for mt in range(2):
                        msl = slice(mt * 512, (mt + 1) * 512)
                        op_ps = psd.tile([128, 512], F32, tag="op")
                        nc.tensor.matmul(op_ps[:], oT[0][:, rsl],
                                         wo_t[:, 0, msl], start=True,
                                         stop=False, skip_group_check=True)
                        nc.tensor.matmul(op_ps[:], oT[1][:, rsl],
                                         wo_t[:, 1, msl], start=False,
                                         stop=True, skip_group_check=True)
                        ost = pd.tile([128, 512], F32, tag="ost")
                        nc.vector.tensor_copy(ost[:], op_ps[:])
                        nc.sync.dma_start(out_ext[rsl, msl], ost[:])
            pbc_cm.__exit__(None, None, None)

    nc.compile()
    return nc


_NC_CACHE = {}


def _get_nc():
    if "nc" not in _NC_CACHE:
        _NC_CACHE["nc"] = _build()
    return _NC_CACHE["nc"]


def _prep_in_maps(x, ln_w, ln_b, w_qkv, w_out):
    import ml_dtypes
    _bf = ml_dtypes.bfloat16
    x = np.asarray(x, dtype=np.float32)
    ln_w = np.asarray(ln_w, dtype=np.float32)
    ln_b = np.asarray(ln_b, dtype=np.float32)
    w_qkv = np.asarray(w_qkv, dtype=np.float32)
    w_out = np.asarray(w_out, dtype=np.float32)

    ones = np.ones((128, 128), dtype=np.float32)
    # mask[jp, ii] = 1 iff jp <= ii  (keep j <= i)
    mask = np.triu(np.ones((128, 128), dtype=ml_dtypes.bfloat16))
    ident = np.eye(128, dtype=np.float32)

    xTs = [x[b].T.astype(_bf) for b in range(B)]

    in_maps = []
    for core in range(8):
        b, hg = core // 4, core % 4
        csl = slice(hg * CD, (hg + 1) * CD)
        # raw slices with SCALE folded into q
        w0 = np.concatenate([w_qkv[:, csl] * SCALE,
                             w_qkv[:, DIM + hg * CD:DIM + (hg + 1) * CD],
                             w_qkv[:, 2 * DIM + hg * CD:2 * DIM + (hg + 1) * CD]],
                            axis=1)
        wf = ln_w[:, None] * w0                      # ln_w folded
        u = wf.sum(axis=0)                           # pairs with -mean
        vb = ln_b @ w0                               # pairs with std (ln bias)
        uv = np.stack([u, vb]).astype(np.float32)
        in_maps.append({
            "xT": xTs[b],
            "wqkv": wf.astype(_bf),
            "uv": _r32(uv),
            "wout": w_out[csl, :].astype(_bf),
            "ones": ones,
            "mask": mask,
            "ident": ident.astype(_bf),
        })
    return in_maps


def _combine(results):
    out = np.empty((B, N, DIM), dtype=np.float32)
    for b in range(B):
        acc = results[b * 4]["out"].astype(np.float32)
        for hg in range(1, 4):
            acc = acc + results[b * 4 + hg]["out"]
        out[b] = acc
    return out


def kernel(x, ln_w, ln_b, w_qkv, w_out):
    nc = _get_nc()
    in_maps = _prep_in_maps(x, ln_w, ln_b, w_qkv, w_out)
    res = run_bass_kernel_spmd(nc, in_maps, core_ids=list(range(8)))
    return _combine(res.results)


def run_traced(x, ln_w, ln_b, w_qkv, w_out, **kwargs):
    """Run with NTFF profiling; returns (output, BassKernelResults)."""
    nc = _get_nc()
    in_maps = _prep_in_maps(x, ln_w, ln_b, w_qkv, w_out)
    res = run_bass_kernel_spmd(nc, in_maps, core_ids=list(range(8)),
                               trace=True, **kwargs)
    return _combine(res.results), res
